# revision 6
# baseline (speedup 1.0000x reference)
"""Trainium2 Bass kernel for nn_DualSPRTLinear: out = x @ (ternary*scales).T

Shapes:
  x       [4, 2048, 4096] fp32  -> tokens T=8192, contraction K=4096
  ternary [4096, 4096]    int8  (out-features O x K, values {-1,0,1})
  scales  [131072]        fp32  one scale per contiguous 128-weight group
  out     [4, 2048, 4096] fp32

Strategy: data-parallel over tokens on 8 cores (TC=1024/core).  The PE
issues one matmul every ~216 ns regardless of dtype; a bf16 matmul
contracts 128 rows, an fp8 DoubleRow matmul contracts 256 — 2x.  Full
fp8 fails the 2e-2 gate (e4m3 has 3 mantissa bits), so K is split:
22 chunks run in bf16 (exact path), 10 chunks run as 5 fp8 DoubleRow
pairs.  The fp8 chunks carry a per-out-feature prescale C_o (chosen on
host from 64 candidates to minimize e4m3 scale-quantization error);
the bf16 weights carry 16*C_o too, so all 27 matmuls of a chain
accumulate in one PSUM bank, and a single fp32 row-multiply
(1/(16*C_o), on VectorE) undoes it at eviction.  Host-simulated
rel_absmax = 0.0179 (gate 2e-2).

Per (j, m) chain: 22 bf16 + 5 DR matmuls = 27 instrs ~ 5.8us;
64 chains -> ~373us PE, vs 469us bf16 baseline.
"""

import os
import sys

import numpy as np

for _p in ("/opt/trn_rl_repo",):
    if _p not in sys.path and os.path.isdir(_p):
        sys.path.append(_p)

import ml_dtypes

import concourse.bacc as bacc
import concourse.mybir as mybir
import concourse.tile as tile
from concourse.bass_utils import run_bass_kernel_spmd

BF16 = ml_dtypes.bfloat16
E4M3 = ml_dtypes.float8_e4m3

_AXON_SO = "/opt/axon/libaxon_pjrt.so"


def _ensure_ntff_hook():
    """Recreate the antenv.axon_hooks module + NTFF hook via ctypes on the
    axon PJRT .so (the agent image lacks axon_hooks)."""
    import types

    if "antenv.axon_hooks" in sys.modules:
        return
    import contextlib
    import ctypes

    import antenv

    mod = types.ModuleType("antenv.axon_hooks")
    _state = {"hook": None}
    mod.set_axon_ntff_profile_hook = lambda h: _state.__setitem__("hook", h)
    mod.get_axon_ntff_profile_hook = lambda: _state["hook"]
    sys.modules["antenv.axon_hooks"] = mod
    antenv.axon_hooks = mod

    if not os.path.exists(_AXON_SO):
        return
    lib = ctypes.CDLL(_AXON_SO)
    if not hasattr(lib, "axon_start_nrt_profile"):
        return
    lib.axon_start_nrt_profile.argtypes = [
        ctypes.POINTER(ctypes.c_int64),
        ctypes.c_size_t,
    ]
    lib.axon_start_nrt_profile.restype = ctypes.c_int64
    lib.axon_stop_nrt_profile.argtypes = [ctypes.c_char_p]
    lib.axon_stop_nrt_profile.restype = ctypes.c_int64

    @contextlib.contextmanager
    def _hook(output_dir, device_ids):
        import jax

        jax.devices()
        if device_ids:
            ids = (ctypes.c_int64 * len(device_ids))(*device_ids)
            rc = lib.axon_start_nrt_profile(ids, len(device_ids))
        else:
            rc = lib.axon_start_nrt_profile(None, 0)
        if rc != 0:
            raise RuntimeError(f"axon_start_nrt_profile rc={rc}")
        try:
            yield
        finally:
            n = lib.axon_stop_nrt_profile(str(output_dir).encode())
            print(f"profile: {n} file(s) written to {output_dir}", file=sys.stderr)

    _state["hook"] = _hook


N_CORES = 8
T = 8192
TC = T // N_CORES     # 1024 tokens/core
K = 4096
O = 4096
GS = 128
NG = K // GS          # 32 k-chunks
NB = 22               # bf16 chunks
N8 = NG - NB          # 10 fp8 chunks
ND = N8 // 2          # 5 DoubleRow pair-tiles
OB = 512              # o-block (psum free dim)
NJ = O // OB          # 8
NM = TC // 128        # 8 token blocks
BW = (4, 4, 6, 8)     # bf16 super-tile widths (sum = NB)
BOFF = (0, 4, 8, 14)  # chunk offset of each super-tile


def _build():
    nc = bacc.Bacc(None, target_bir_lowering=False, debug=False)
    xb = nc.dram_tensor("xb", [128, NB, TC], mybir.dt.bfloat16, kind="ExternalInput")
    x8 = nc.dram_tensor("x8", [128, N8, TC], mybir.dt.float8e4, kind="ExternalInput")
    wb = nc.dram_tensor("wb", [NJ, 128, NB, OB], mybir.dt.bfloat16, kind="ExternalInput")
    w8 = nc.dram_tensor("w8", [NJ, 128, ND, 2, OB], mybir.dt.float8e4, kind="ExternalInput")
    cr = nc.dram_tensor("cr", [NJ, 128, OB], mybir.dt.float32, kind="ExternalInput")
    out = nc.dram_tensor("out", [TC, O], mybir.dt.bfloat16, kind="ExternalOutput")

    DR = mybir.MatmulPerfMode.DoubleRow

    with tile.TileContext(nc) as tc:
        with (
            tc.tile_pool(name="xres", bufs=1) as xpool,
            tc.tile_pool(name="x8res", bufs=1) as x8pool,
            tc.tile_pool(name="crow", bufs=1) as cpool,
            tc.tile_pool(name="wbuf", bufs=48) as wpool,
            tc.tile_pool(name="w8buf", bufs=3) as w8pool,
            tc.tile_pool(name="ostg", bufs=12) as opool,
            tc.tile_pool(name="warm", bufs=1) as warmpool,
            tc.tile_pool(name="psum", bufs=8, space="PSUM") as ppool,
        ):
            # ---- resident x: one tile per k-chunk (fine-grained arrival) ----
            x_t = [xpool.tile([128, TC], mybir.dt.bfloat16, name=f"x_{g}")
                   for g in range(NB)]
            x8_t = [x8pool.tile([128, 2, TC], mybir.dt.float8e4, name=f"x8_{d}")
                    for d in range(ND)]
            cr_sb = cpool.tile([128, NJ, OB], mybir.dt.float32, name="cr")

            # DMA choreography (arrival = per-ring emission order):
            #   scalar: x chunks 0..13, x8 pairs, corow slices, then out-DMAs
            #   sync:   W j0 chunks, x chunks 14..21, w8 j0, then j1, j2, ...
            XSPLIT = 14
            for g in range(XSPLIT):
                nc.scalar.dma_start(x_t[g][:], xb[:, g, :])

            def issue_w(j):
                # j>0: w8 first so the DR-leading section finds it resident
                dts = None
                if j > 0:
                    dts = w8pool.tile([128, ND, 2, OB], mybir.dt.float8e4,
                                      name=f"w8_{j}", tag="w8")
                    nc.sync.dma_start(dts[:], w8[j])
                tiles = []
                for g in range(NB):
                    t = wpool.tile([128, OB], mybir.dt.bfloat16,
                                   name=f"wb_{j}_{g}", tag="wb")
                    nc.sync.dma_start(t[:], wb[j, :, g, :])
                    tiles.append(t)
                    if j == 0 and g == NB - 1:
                        for g2 in range(XSPLIT, NB):
                            nc.sync.dma_start(x_t[g2][:], xb[:, g2, :])
                if j == 0:
                    dts = w8pool.tile([128, ND, 2, OB], mybir.dt.float8e4,
                                      name=f"w8_{j}", tag="w8")
                    nc.sync.dma_start(dts[:], w8[j])
                return tiles, dts

            w_tiles = {}
            w_tiles[0] = issue_w(0)
            for d in range(ND):
                nc.scalar.dma_start(x8_t[d][:], x8[:, 2 * d : 2 * d + 2, :])
            for j in range(NJ):
                nc.scalar.dma_start(cr_sb[:, j, :], cr[j])

            # PE warm-up: covers engine preamble until first x/W tiles land
            warm_sb = warmpool.tile([128, OB], mybir.dt.bfloat16)
            nc.vector.memset(warm_sb[:], 0.0)
            warm_ps = ppool.tile([128, OB], mybir.dt.float32, name="ps_warm", tag="ps")
            for i in range(6):
                nc.tensor.matmul(
                    warm_ps[:], warm_sb[:, :128], warm_sb[:], start=True, stop=True
                )

            for j in range(NJ):
                tiles, dts = w_tiles.pop(j)
                if j + 1 < NJ:
                    w_tiles[j + 1] = issue_w(j + 1)
                psum_tiles = [
                    ppool.tile([128, OB], mybir.dt.float32, name=f"ps_{j}_{m}", tag="ps")
                    for m in range(NM)
                ]

                def emit_bf16(m_range, start, stop):
                    for g in range(NB):
                        for m in m_range:
                            nc.tensor.matmul(
                                psum_tiles[m][:],
                                x_t[g][:, m * 128 : (m + 1) * 128],
                                tiles[g][:],
                                start=(start and g == 0),
                                stop=(stop and g == NB - 1),
                            )

                def emit_dr(m_range, start, stop, m_outer):
                    order = (
                        [(m, d) for m in m_range for d in range(ND)]
                        if m_outer
                        else [(m, d) for d in range(ND) for m in m_range]
                    )
                    for m, d in order:
                        nc.tensor.matmul(
                            psum_tiles[m][:],
                            x8_t[d][:, :, m * 128 : (m + 1) * 128],
                            dts[:, d],
                            start=(start and d == 0),
                            stop=(stop and d == ND - 1),
                            perf_mode=DR,
                        )

                def evict(m, last_j):
                    o_t = opool.tile([128, OB], mybir.dt.bfloat16,
                                     name=f"o_{j}_{m}", tag="o")
                    nc.vector.tensor_tensor(
                        o_t[:], psum_tiles[m][:], cr_sb[:, j, :], mybir.AluOpType.mult
                    )
                    oeng = nc.sync if (last_j and m % 2 == 1) else nc.scalar
                    oeng.dma_start(
                        out[m * 128 : (m + 1) * 128, j * OB : (j + 1) * OB], o_t[:]
                    )

                if j == 0:
                    # x still streaming: chunk-major bf16 then DR, stop on DR
                    emit_bf16(range(NM), start=True, stop=False)
                    emit_dr(range(NM), start=False, stop=True, m_outer=False)
                    for m in range(NM):
                        evict(m, False)
                else:
                    # DR-leading: j-1 ended on DR mms, so no mode re-entry here;
                    # m-outer order gives prior-j psum evictions time to clear
                    emit_dr(range(NM), start=True, stop=False, m_outer=True)
                    halves = (
                        (range(0, 7), range(7, NM))
                        if j == NJ - 1
                        else (range(0, NM // 2), range(NM // 2, NM))
                    )
                    for half in halves:
                        emit_bf16(half, start=False, stop=True)
                        for m in half:
                            evict(m, j == NJ - 1)

    nc.compile()
    return nc


_NC = None


def _get_nc():
    global _NC
    if _NC is None:
        _NC = _build()
    return _NC


def _q8(a):
    return a.astype(E4M3).astype(np.float32)


def _prep_weights(ternary, scales):
    tern = np.asarray(ternary).astype(np.float32)
    S = np.asarray(scales).astype(np.float32).reshape(O, NG)
    S8 = S[:, NB:]

    # per-out-feature prescale over the fp8 chunks (64 candidates in [1,2))
    best = np.ones(O, dtype=np.float32)
    bcost = np.full(O, np.inf, dtype=np.float32)
    for Cv in (2.0 ** (np.arange(64) / 64)).astype(np.float32):
        d = _q8(16.0 * Cv * S8) / Cv - 16.0 * S8
        cost = (d * d).sum(axis=1)
        sel = cost < bcost
        best[sel] = Cv
        bcost[sel] = cost[sel]
    C = best

    w_full = tern * S.repeat(GS, axis=1)                       # [O, K]
    KB = NB * GS
    wb_host = (16.0 * C[:, None] * w_full[:, :KB]).astype(BF16)
    wb_host = np.ascontiguousarray(
        wb_host.reshape(NJ, OB, NB, 128).transpose(0, 3, 2, 1)
    )  # [j, p, g, oi]

    s8q = _q8(16.0 * C[:, None] * S8)                          # [O, N8] e4m3 grid
    w8_host = (tern[:, KB:] * s8q.repeat(GS, axis=1)).astype(E4M3)  # exact in e4m3
    w8_host = np.ascontiguousarray(
        w8_host.reshape(NJ, OB, ND, 2, 128).transpose(0, 4, 2, 3, 1)
    )  # [j, p, d, two, oi]

    cr_host = (1.0 / (16.0 * C)).astype(np.float32).reshape(NJ, 1, OB)
    cr_host = np.ascontiguousarray(np.broadcast_to(cr_host, (NJ, 128, OB)))
    return wb_host, w8_host, cr_host


def _prep_inputs(x, ternary, scales):
    x = np.asarray(x)
    wb_host, w8_host, cr_host = _prep_weights(ternary, scales)

    xt = x.reshape(T, K)
    KB = NB * GS
    in_maps = []
    for c in range(N_CORES):
        xcT = np.ascontiguousarray(xt[c * TC : (c + 1) * TC].T)  # [K, TC] fp32
        xb_c = np.ascontiguousarray(
            xcT[:KB].reshape(NB, 128, TC).transpose(1, 0, 2).astype(BF16)
        )
        x8_c = np.ascontiguousarray(
            xcT[KB:].reshape(N8, 128, TC).transpose(1, 0, 2).astype(E4M3)
        )
        in_maps.append(
            {"xb": xb_c, "x8": x8_c, "wb": wb_host, "w8": w8_host, "cr": cr_host}
        )
    return in_maps


def run(x, ternary, scales, trace=False, **trace_kwargs):
    """Run on 8 NeuronCores; returns (out [4,2048,4096] fp32, BassKernelResults)."""
    nc = _get_nc()
    if trace:
        _ensure_ntff_hook()
    in_maps = _prep_inputs(x, ternary, scales)
    res = run_bass_kernel_spmd(
        nc, in_maps, core_ids=list(range(N_CORES)), trace=trace, **trace_kwargs
    )
    parts = [np.asarray(r["out"]) for r in res.results]
    out = np.concatenate(parts, axis=0).astype(np.float32).reshape(4, 2048, O)
    return out, res


def kernel(x, ternary, scales):
    out, _ = run(x, ternary, scales, trace=False)
    return out
